# revision 5
# baseline (speedup 1.0000x reference)
"""Trainium2 Bass kernel for nn_CA_82300163326040.

Cross-attention between X and Y with softmax over the BATCH axis (torch
legacy dim=0). B=8, N=2048, D=512, f32.

Sharding: data-parallel over batch, one batch per NeuronCore (8 cores).
The batch-axis softmax couples cores: each core computes its local
shifted exp-scores Es = exp(Q.K^T*s) - c ([2048,2048], fp8e4m3) and the
denominators Z = sum_b E are obtained with 8-core fp8 AllReduces of Es
(Z = AR(Es) + 8c reconstructed on the fly). The shift c ~ E[exp(S)]
keeps |Es| ~3x smaller than E so fp8 quantization + the mesh CCE's
fp8 partial-sum roundings stay small. No max-subtraction is needed:
S ~ N(0, 1/9) so exp() cannot overflow.

v3 (vs 579us baseline / 547us v2a):
  - Q/K projections emit fp8e4m3 (D^-1/4 folded into both Q and K
    weights so values sit in fp8's normal range); score matmuls run
    fp8 DoubleRow (2 k-subtiles per instruction, ~1.7x).
  - Single fp8 score tensor: AR input IS the stored numerator; phase C
    reconstructs A = (Es + c) * (1/Z) in one fused DVE
    scalar_tensor_tensor. No bf16 E round-trip (saves 32MB DMA).
  - 4-bank-wide PSUM tiles ([128,2048]) everywhere: 4x fewer ACT
    evictions/DMAs, each amortizing the 352-cycle ACT ramp.
  - X+Y residual accumulated in PSUM via identity matmuls; bf16 output.
  - Z fp8->f32 (+8c) conversion on the otherwise-idle GpSimd engine.
  - E1 scores + their ARs issue right after the Q1/K2 projections, so
    all four AllReduces hide under compute.
"""

import numpy as np

import concourse.bass as bass
import concourse.mybir as mybir
import concourse.tile as tile
from concourse import bacc
from concourse.bass_utils import run_bass_kernel_spmd

P = 128
N = 2048  # sequence length
D = 512  # model dim
NCORES = 8
DT = D // P  # 4 feature tiles
NT = N // P  # 16 sequence tiles
CH = 512  # n-chunk (free dim of all matmuls)
NCH = N // CH  # 4 chunks
HT = NT // 2  # half of the m-tiles (phase-C staging granularity)
WF = 4 * CH  # wide psum tile free size (4 banks)

F32 = mybir.dt.float32
BF16 = mybir.dt.bfloat16
F8 = mybir.dt.float8e4

C_SHIFT = 1.0546875  # ~ E[exp(S)]; Es = E - c
Z_BIAS = 8 * C_SHIFT  # Z = AR(Es) + 8c

_CACHE = {}


def build():
    nc = bacc.Bacc("TRN2", target_bir_lowering=False, debug=False, num_devices=NCORES)

    # ---- parameters (per core), all pre-arranged on host ----
    xtb = nc.declare_dram_parameter("XTB", [P, DT, N], BF16, isOutput=False)
    ytb = nc.declare_dram_parameter("YTB", [P, DT, N], BF16, isOutput=False)
    # weights (transposed, partition-major): w[p, o, e] = W^T[o*128+p, e]
    w_q1 = nc.declare_dram_parameter("WQ1T", [P, DT, D], BF16, isOutput=False)
    w_k1 = nc.declare_dram_parameter("WK1T", [P, DT, D], BF16, isOutput=False)
    w_v1 = nc.declare_dram_parameter("WV1T", [P, DT, D], BF16, isOutput=False)
    w_q2 = nc.declare_dram_parameter("WQ2T", [P, DT, D], BF16, isOutput=False)
    w_k2 = nc.declare_dram_parameter("WK2T", [P, DT, D], BF16, isOutput=False)
    w_v2 = nc.declare_dram_parameter("WV2T", [P, DT, D], BF16, isOutput=False)
    b_q1 = nc.declare_dram_parameter("BQ1", [P, DT], F32, isOutput=False)
    b_k1 = nc.declare_dram_parameter("BK1", [P, DT], F32, isOutput=False)
    b_q2 = nc.declare_dram_parameter("BQ2", [P, DT], F32, isOutput=False)
    b_k2 = nc.declare_dram_parameter("BK2", [P, DT], F32, isOutput=False)
    b_v1 = nc.declare_dram_parameter("BV1", [P, 4, D], BF16, isOutput=False)  # x4 tiled
    b_v2 = nc.declare_dram_parameter("BV2", [P, 4, D], BF16, isOutput=False)
    idn = nc.declare_dram_parameter("IDN", [P, P], BF16, isOutput=False)

    out = nc.declare_dram_parameter("OT", [DT, NCH, P, CH], BF16, isOutput=True)

    with tile.TileContext(nc) as tc:
        with (
            tc.tile_pool(name="w", bufs=2) as p_w,
            tc.tile_pool(name="bias", bufs=1) as p_bias,
            tc.tile_pool(name="inp", bufs=2) as p_in,
            tc.tile_pool(name="qk", bufs=2) as p_qk,
            tc.tile_pool(name="v", bufs=2) as p_v,
            tc.tile_pool(name="zf", bufs=1) as p_zf,
            tc.tile_pool(name="rz", bufs=1) as p_rz,
            tc.tile_pool(name="zb", bufs=2) as p_zb,
            tc.tile_pool(name="ere", bufs=2) as p_ere,
            tc.tile_pool(name="a", bufs=4) as p_a,
            tc.tile_pool(name="esb", bufs=2) as p_esb,
            tc.tile_pool(name="es8", bufs=2) as p_es8,
            tc.tile_pool(name="ot", bufs=2) as p_ot,
            tc.tile_pool(name="ps", bufs=2, space="PSUM") as p_ps,
            tc.tile_pool(name="dram", bufs=1, space="DRAM") as p_dram,
        ):
            # ---- DRAM intermediates: shifted fp8 scores + AR outputs ----
            e1s_d = p_dram.tile([NCH, NT, P, CH], F8, tag="e1s")
            e2s_d = p_dram.tile([NCH, NT, P, CH], F8, tag="e2s")
            z1_h = [
                p_dram.tile([2, NT, P, CH], F8, tag=f"z1{h}",
                            addr_space="Shared", name=f"z1{h}")
                for h in range(2)
            ]
            z2_h = [
                p_dram.tile([2, NT, P, CH], F8, tag=f"z2{h}",
                            addr_space="Shared", name=f"z2{h}")
                for h in range(2)
            ]

            # ---- resident loads; first-proj operands issued first ----
            w1_sb = p_w.tile([P, DT, D], BF16, tag="w", name="wq1")
            nc.sync.dma_start(w1_sb[:], w_q1[:])
            xt_sb = p_in.tile([P, DT, N], BF16, tag="in", name="xt")
            nc.sync.dma_start(xt_sb[:, :, 0:1024], xtb[:, :, 0:1024])
            nc.sync.dma_start(xt_sb[:, :, 1024:2048], xtb[:, :, 1024:2048])
            w2_sb = p_w.tile([P, DT, D], BF16, tag="w", name="wk2")
            nc.sync.dma_start(w2_sb[:], w_k2[:])
            yt_sb = p_in.tile([P, DT, N], BF16, tag="in", name="yt")
            nc.sync.dma_start(yt_sb[:, :, 0:1024], ytb[:, :, 0:1024])
            nc.sync.dma_start(yt_sb[:, :, 1024:2048], ytb[:, :, 1024:2048])

            bq1_sb = p_bias.tile([P, DT], F32, tag="bq1")
            bk1_sb = p_bias.tile([P, DT], F32, tag="bk1")
            bq2_sb = p_bias.tile([P, DT], F32, tag="bq2")
            bk2_sb = p_bias.tile([P, DT], F32, tag="bk2")
            bv1_sb = p_bias.tile([P, 4, D], BF16, tag="bv1")
            bv2_sb = p_bias.tile([P, 4, D], BF16, tag="bv2")
            idn_sb = p_bias.tile([P, P], BF16, tag="idn")
            nc.sync.dma_start(bq1_sb[:], b_q1[:])
            nc.sync.dma_start(bk2_sb[:], b_k2[:])
            nc.sync.dma_start(bk1_sb[:], b_k1[:])
            nc.sync.dma_start(bq2_sb[:], b_q2[:])
            nc.sync.dma_start(bv1_sb[:], b_v1[:])
            nc.sync.dma_start(bv2_sb[:], b_v2[:])
            nc.sync.dma_start(idn_sb[:], idn[:])

            def load_w(wp, name):
                w_sb = p_w.tile([P, DT, D], BF16, tag="w", name=name)
                nc.sync.dma_start(w_sb[:], wp[:])
                return w_sb

            def proj_T(w_sb, src_sb, bias_sb, name, odt):
                """out[e, n] = sum_d W[e,d] src[n,d] + b[e], e-major."""
                o_sb = p_qk.tile([P, DT, N], odt, tag="qk", name=name)
                for eo in range(DT):
                    ps = p_ps.tile([P, 4, CH], F32, tag="ps")
                    for do in range(DT):
                        for ch in range(NCH):
                            nc.tensor.matmul(
                                ps[:, ch, :],
                                w_sb[:, do, eo * P : (eo + 1) * P],
                                src_sb[:, do, ch * CH : (ch + 1) * CH],
                                start=(do == 0),
                                stop=(do == DT - 1),
                            )
                    nc.scalar.activation(
                        o_sb[:, eo, :].rearrange("p (m c) -> p m c", m=4),
                        ps[:],
                        mybir.ActivationFunctionType.Identity,
                        bias=bias_sb[:, eo : eo + 1],
                    )
                return o_sb

            def proj_V(w_sb, src_sb, bias_sb, name):
                """out[m, e] = sum_d src[m,d] W[e,d] + b[e], m-major bf16."""
                o_sb = p_v.tile([P, NT, D], BF16, tag="v", name=name)
                for mq in range(NT // 4):
                    ps = p_ps.tile([P, 4, CH], F32, tag="ps")
                    for mi in range(4):
                        mt = 4 * mq + mi
                        for do in range(DT):
                            nc.tensor.matmul(
                                ps[:, mi, :],
                                src_sb[:, do, mt * P : (mt + 1) * P],
                                w_sb[:, do, :],
                                start=(do == 0),
                                stop=(do == DT - 1),
                            )
                    nc.vector.tensor_add(
                        out=o_sb[:, 4 * mq : 4 * mq + 4, :],
                        in0=ps[:],
                        in1=bias_sb[:],
                    )
                return o_sb

            DR = mybir.MatmulPerfMode.DoubleRow

            def scores_exp(kt_sb, qt_sb, es_dram, chs):
                """Es[ch, mt, p, c] = exp(sum_e K[m,e] Q[n,e]) - c -> fp8 DRAM."""
                for ch in chs:
                    csl = slice(ch * CH, (ch + 1) * CH)
                    for mq in range(NT // 4):
                        ps = p_ps.tile([P, 4, CH], F32, tag="ps")
                        for mi in range(4):
                            mt = 4 * mq + mi
                            for eo in range(0, DT, 2):
                                nc.tensor.matmul(
                                    ps[:, mi, :],
                                    kt_sb[:, eo : eo + 2, mt * P : (mt + 1) * P],
                                    qt_sb[:, eo : eo + 2, csl],
                                    start=(eo == 0),
                                    stop=(eo == DT - 2),
                                    perf_mode=DR,
                                )
                        e_sb = p_esb.tile([P, 4, CH], BF16, tag="esb")
                        nc.scalar.activation(
                            e_sb[:], ps[:], mybir.ActivationFunctionType.Exp
                        )
                        es_sb = p_es8.tile([P, 4, CH], F8, tag="es8")
                        nc.vector.tensor_scalar_sub(
                            out=es_sb[:], in0=e_sb[:], scalar1=C_SHIFT
                        )
                        nc.sync.dma_start(
                            es_dram[ch, 4 * mq : 4 * mq + 4].rearrange(
                                "m p c -> p m c"
                            ),
                            es_sb[:],
                        )

            def ar_half(es_d, z_halves, h):
                sl = slice(2 * h, 2 * h + 2)
                nc.gpsimd.collective_compute(
                    "AllReduce",
                    mybir.AluOpType.add,
                    replica_groups=[list(range(NCORES))],
                    ins=[es_d[sl].opt()],
                    outs=[z_halves[h][:].opt()],
                )

            # ======== branch 1: Q1/K2 projections, E1 scores, ARs ========
            q1t = proj_T(w1_sb, xt_sb, bq1_sb, "q1t", F8)
            k2t = proj_T(w2_sb, yt_sb, bk2_sb, "k2t", F8)
            scores_exp(k2t, q1t, e1s_d, (0, 1))
            ar_half(e1s_d, z1_h, 0)
            scores_exp(k2t, q1t, e1s_d, (2, 3))
            ar_half(e1s_d, z1_h, 1)

            # ======== branch 2: K1/Q2 projections, E2 scores, ARs ========
            w_sb = load_w(w_k1, "wk1")
            k1t = proj_T(w_sb, xt_sb, bk1_sb, "k1t", F8)
            w_sb = load_w(w_q2, "wq2")
            q2t = proj_T(w_sb, yt_sb, bq2_sb, "q2t", F8)
            scores_exp(k1t, q2t, e2s_d, (0, 1))
            ar_half(e2s_d, z2_h, 0)
            scores_exp(k1t, q2t, e2s_d, (2, 3))
            ar_half(e2s_d, z2_h, 1)

            # ======== V projections ========
            w_sb = load_w(w_v2, "wv2")
            v2 = proj_V(w_sb, yt_sb, bv2_sb, "v2")
            w_sb = load_w(w_v1, "wv1")
            v1 = proj_V(w_sb, xt_sb, bv1_sb, "v1")

            # ======== phase C ========
            def make_a_half(es_d, z_src, ch, h, name):
                """A[:, 8 mt, c] = (Es + c)/Z for chunk ch, half h; bf16."""
                msl = slice(h * HT, (h + 1) * HT)
                z_b = p_zb.tile([P, HT, CH], F8, tag="zb", name=f"zb{name}")
                nc.sync.dma_start(z_b[:], z_src(ch, msl))
                z_f = p_zf.tile([P, HT, CH], F32, tag="zf", name=f"zf{name}")
                nc.gpsimd.tensor_scalar_add(out=z_f[:], in0=z_b[:], scalar1=Z_BIAS)
                rz = p_rz.tile([P, HT, CH], F32, tag="rz", name=f"rz{name}")
                nc.vector.reciprocal_approx_fast(out=rz[:], in_=z_f[:])
                e_b = p_ere.tile([P, HT, CH], F8, tag="ere", name=f"eb{name}")
                nc.sync.dma_start(e_b[:], es_d[ch, msl].rearrange("m p c -> p m c"))
                a_sb = p_a.tile([P, HT, CH], BF16, tag="a", name=f"a{name}")
                nc.vector.scalar_tensor_tensor(
                    out=a_sb[:],
                    in0=e_b[:],
                    scalar=C_SHIFT,
                    in1=rz[:],
                    op0=mybir.AluOpType.add,
                    op1=mybir.AluOpType.mult,
                )
                return a_sb

            def z1_src(ch, msl):
                return z1_h[ch // 2][ch % 2, msl].rearrange("m p c -> p m c")

            def z2_src(ch, msl):
                return z2_h[ch // 2][ch % 2, msl].rearrange("m p c -> p m c")

            ps_held = {}

            def u1_pass(ch):
                a1 = [make_a_half(e1s_d, z1_src, ch, h, f"1{h}") for h in range(2)]
                ps = p_ps.tile([P, 4, CH], F32, tag="ps")
                for dt in range(DT):
                    dsl = slice(dt * P, (dt + 1) * P)
                    for mt in range(NT):
                        nc.tensor.matmul(
                            ps[:, dt, :],
                            v2[:, mt, dsl],
                            a1[mt // HT][:, mt % HT, :],
                            start=(mt == 0),
                            stop=False,
                        )
                ps_held[ch] = ps

            def u2_pass(ch):
                a2 = [make_a_half(e2s_d, z2_src, ch, h, f"2{h}") for h in range(2)]
                csl = slice(ch * CH, (ch + 1) * CH)
                ps = ps_held[ch]
                for dt in range(DT):
                    dsl = slice(dt * P, (dt + 1) * P)
                    psd = ps[:, dt, :]
                    for mt in range(NT):
                        nc.tensor.matmul(
                            psd,
                            v1[:, mt, dsl],
                            a2[mt // HT][:, mt % HT, :],
                            start=False,
                            stop=False,
                        )
                    # residual: out += I.T @ (X^T + Y^T) chunk
                    nc.tensor.matmul(psd, idn_sb[:], xt_sb[:, dt, csl],
                                     start=False, stop=False)
                    nc.tensor.matmul(psd, idn_sb[:], yt_sb[:, dt, csl],
                                     start=False, stop=True)
                ot = p_ot.tile([P, 4, CH], BF16, tag="ot")
                nc.scalar.activation(
                    ot[:], ps[:], mybir.ActivationFunctionType.Copy
                )
                nc.sync.dma_start(
                    out[:, ch].rearrange("d p c -> p d c"), ot[:]
                )

            u1_pass(0)
            u1_pass(1)
            u2_pass(0)
            u1_pass(2)
            u2_pass(1)
            u1_pass(3)
            u2_pass(2)
            u2_pass(3)

    nc.compile()
    return nc


def _pmajor(a, inner):
    """[O*P, F] -> [P, O, F] partition-major."""
    o = a.shape[0] // inner
    return np.ascontiguousarray(a.reshape(o, inner, a.shape[1]).transpose(1, 0, 2))


def _prep_inputs(inputs):
    import ml_dtypes

    X = np.asarray(inputs["X"], dtype=np.float32)
    Y = np.asarray(inputs["Y"], dtype=np.float32)
    shalf = np.float32(1.0 / np.sqrt(np.sqrt(D)))  # D^-1/4 into both Q and K

    def wT(name, s=np.float32(1.0)):
        w = np.asarray(inputs[f"W_{name}"], dtype=np.float32)
        return _pmajor((w.T * s).astype(ml_dtypes.bfloat16), P)

    def bstripe(name, s=np.float32(1.0)):
        b = np.asarray(inputs[f"b_{name}"], dtype=np.float32) * s
        return np.ascontiguousarray(b.reshape(DT, P).T)

    def bbcast4(name):
        b = np.asarray(inputs[f"b_{name}"], dtype=np.float32).astype(ml_dtypes.bfloat16)
        return np.ascontiguousarray(np.broadcast_to(b, (P, 4, D)))

    shared = {
        "WQ1T": wT("xq", shalf),
        "WK1T": wT("xk", shalf),
        "WV1T": wT("xv"),
        "WQ2T": wT("yq", shalf),
        "WK2T": wT("yk", shalf),
        "WV2T": wT("yv"),
        "BQ1": bstripe("xq", shalf),
        "BK1": bstripe("xk", shalf),
        "BQ2": bstripe("yq", shalf),
        "BK2": bstripe("yk", shalf),
        "BV1": bbcast4("xv"),
        "BV2": bbcast4("yv"),
        "IDN": np.eye(P, dtype=ml_dtypes.bfloat16),
    }
    in_maps = []
    for c in range(NCORES):
        xt = np.ascontiguousarray(X[c].T)
        yt = np.ascontiguousarray(Y[c].T)
        m = dict(shared)
        m["XTB"] = _pmajor(xt.astype(ml_dtypes.bfloat16), P)
        m["YTB"] = _pmajor(yt.astype(ml_dtypes.bfloat16), P)
        in_maps.append(m)
    return in_maps


def _unblock(ot):
    """[DT, NCH, P, CH] -> [N, D] (transposed back)."""
    return ot.transpose(0, 2, 1, 3).reshape(D, N).T


def kernel(**inputs):
    if "nc" not in _CACHE:
        _CACHE["nc"] = build()
    nc = _CACHE["nc"]
    in_maps = _prep_inputs(inputs)
    res = run_bass_kernel_spmd(
        nc, in_maps, core_ids=list(range(NCORES)), **_CACHE.get("run_kwargs", {})
    )
    _CACHE["last_result"] = res
    out = np.stack(
        [np.ascontiguousarray(_unblock(res.results[c]["OT"])) for c in range(NCORES)]
    )
    return out.astype(np.float32)


# revision 7
# speedup vs baseline: 2.6471x; 2.6471x over previous
"""Trainium2 Bass kernel for nn_CA_82300163326040.

Cross-attention between X and Y with softmax over the BATCH axis (torch
legacy dim=0). B=8, N=2048, D=512, f32.

Sharding: data-parallel over batch, one batch per NeuronCore (8 cores).
The batch-axis softmax couples cores: each core computes its local
shifted exp-scores Es = exp(Q.K^T*s) - c ([2048,2048], fp8e4m3) and the
denominators Z = sum_b E are obtained with 8-core fp8 AllReduces of Es
(Z = AR(Es) + 8c reconstructed on the fly). The shift c ~ E[exp(S)]
keeps |Es| ~3x smaller than E so fp8 quantization + the mesh CCE's
fp8 partial-sum roundings stay small. No max-subtraction is needed:
S ~ N(0, 1/9) so exp() cannot overflow.

v3 (vs 579us baseline / 547us v2a):
  - Q/K projections emit fp8e4m3 (D^-1/4 folded into both Q and K
    weights so values sit in fp8's normal range); score matmuls run
    fp8 DoubleRow (2 k-subtiles per instruction, ~1.7x).
  - Single fp8 score tensor: AR input IS the stored numerator; phase C
    reconstructs A = (Es + c) * (1/Z) in one fused DVE
    scalar_tensor_tensor. No bf16 E round-trip (saves 32MB DMA).
  - 4-bank-wide PSUM tiles ([128,2048]) everywhere: 4x fewer ACT
    evictions/DMAs, each amortizing the 352-cycle ACT ramp.
  - X+Y residual accumulated in PSUM via identity matmuls; bf16 output.
  - Z fp8->f32 (+8c) conversion on the otherwise-idle GpSimd engine.
  - E1 scores + their ARs issue right after the Q1/K2 projections, so
    all four AllReduces hide under compute.
"""

import numpy as np

import concourse.bass as bass
import concourse.mybir as mybir
import concourse.tile as tile
from concourse import bacc
from concourse.bass_utils import run_bass_kernel_spmd

P = 128
N = 2048  # sequence length
D = 512  # model dim
NCORES = 8
DT = D // P  # 4 feature tiles
NT = N // P  # 16 sequence tiles
CH = 512  # n-chunk (free dim of all matmuls)
NCH = N // CH  # 4 chunks
HT = NT // 2  # half of the m-tiles (phase-C staging granularity)
WF = 4 * CH  # wide psum tile free size (4 banks)

F32 = mybir.dt.float32
BF16 = mybir.dt.bfloat16
F8 = mybir.dt.float8e4

C_SHIFT = 1.0546875  # ~ E[exp(S)]; Es = E - c
Z_BIAS = 8 * C_SHIFT  # Z = AR(Es) + 8c

_CACHE = {}


def build():
    nc = bacc.Bacc("TRN2", target_bir_lowering=False, debug=False, num_devices=NCORES)

    # ---- parameters (per core), all pre-arranged on host ----
    xtb = nc.declare_dram_parameter("XTB", [P, DT, N], BF16, isOutput=False)
    ytb = nc.declare_dram_parameter("YTB", [P, DT, N], BF16, isOutput=False)
    # weights (transposed, partition-major): w[p, o, e] = W^T[o*128+p, e]
    w_q1 = nc.declare_dram_parameter("WQ1T", [P, DT, D], BF16, isOutput=False)
    w_k1 = nc.declare_dram_parameter("WK1T", [P, DT, D], BF16, isOutput=False)
    w_v1 = nc.declare_dram_parameter("WV1T", [P, DT, D], BF16, isOutput=False)
    w_q2 = nc.declare_dram_parameter("WQ2T", [P, DT, D], BF16, isOutput=False)
    w_k2 = nc.declare_dram_parameter("WK2T", [P, DT, D], BF16, isOutput=False)
    w_v2 = nc.declare_dram_parameter("WV2T", [P, DT, D], BF16, isOutput=False)
    b_q1 = nc.declare_dram_parameter("BQ1", [P, DT], F32, isOutput=False)
    b_k1 = nc.declare_dram_parameter("BK1", [P, DT], F32, isOutput=False)
    b_q2 = nc.declare_dram_parameter("BQ2", [P, DT], F32, isOutput=False)
    b_k2 = nc.declare_dram_parameter("BK2", [P, DT], F32, isOutput=False)
    b_v1 = nc.declare_dram_parameter("BV1", [P, 4, D], BF16, isOutput=False)  # x4 tiled
    b_v2 = nc.declare_dram_parameter("BV2", [P, 4, D], BF16, isOutput=False)
    idn = nc.declare_dram_parameter("IDN", [P, P], BF16, isOutput=False)

    out = nc.declare_dram_parameter("OT", [DT, NCH, P, CH], BF16, isOutput=True)

    with tile.TileContext(nc) as tc:
        with (
            tc.tile_pool(name="w", bufs=2) as p_w,
            tc.tile_pool(name="bias", bufs=1) as p_bias,
            tc.tile_pool(name="inp", bufs=2) as p_in,
            tc.tile_pool(name="qk", bufs=2) as p_qk,
            tc.tile_pool(name="v", bufs=2) as p_v,
            tc.tile_pool(name="zf", bufs=1) as p_zf,
            tc.tile_pool(name="rz", bufs=1) as p_rz,
            tc.tile_pool(name="zb", bufs=2) as p_zb,
            tc.tile_pool(name="ere", bufs=2) as p_ere,
            tc.tile_pool(name="a", bufs=3) as p_a,
            tc.tile_pool(name="esb", bufs=2) as p_esb,
            tc.tile_pool(name="es8", bufs=2) as p_es8,
            tc.tile_pool(name="ot", bufs=1) as p_ot,
            tc.tile_pool(name="ps", bufs=2, space="PSUM") as p_ps,
            tc.tile_pool(name="dram", bufs=1, space="DRAM") as p_dram,
        ):
            # ---- DRAM intermediates: shifted fp8 scores + AR outputs ----
            e1s_d = p_dram.tile([NCH, NT, P, CH], F8, tag="e1s")
            e2s_d = p_dram.tile([NCH, NT, P, CH], F8, tag="e2s")
            z1_h = [
                p_dram.tile([2, NT, P, CH], F8, tag=f"z1{h}",
                            addr_space="Shared", name=f"z1{h}")
                for h in range(2)
            ]
            z2_h = [
                p_dram.tile([2, NT, P, CH], F8, tag=f"z2{h}",
                            addr_space="Shared", name=f"z2{h}")
                for h in range(2)
            ]

            # ---- resident loads; first-proj operands issued first ----
            w1_sb = p_w.tile([P, DT, D], BF16, tag="w", name="wq1")
            nc.sync.dma_start(w1_sb[:], w_q1[:])
            xt_sb = p_in.tile([P, DT, N], BF16, tag="in", name="xt")
            nc.sync.dma_start(xt_sb[:, :, 0:1024], xtb[:, :, 0:1024])
            nc.sync.dma_start(xt_sb[:, :, 1024:2048], xtb[:, :, 1024:2048])
            w2_sb = p_w.tile([P, DT, D], BF16, tag="w", name="wk2")
            nc.sync.dma_start(w2_sb[:], w_k2[:])
            yt_sb = p_in.tile([P, DT, N], BF16, tag="in", name="yt")
            nc.sync.dma_start(yt_sb[:, :, 0:1024], ytb[:, :, 0:1024])
            nc.sync.dma_start(yt_sb[:, :, 1024:2048], ytb[:, :, 1024:2048])

            bq1_sb = p_bias.tile([P, DT], F32, tag="bq1")
            bk1_sb = p_bias.tile([P, DT], F32, tag="bk1")
            bq2_sb = p_bias.tile([P, DT], F32, tag="bq2")
            bk2_sb = p_bias.tile([P, DT], F32, tag="bk2")
            bv1_sb = p_bias.tile([P, 4, D], BF16, tag="bv1")
            bv2_sb = p_bias.tile([P, 4, D], BF16, tag="bv2")
            idn_sb = p_bias.tile([P, P], BF16, tag="idn")
            nc.sync.dma_start(bq1_sb[:], b_q1[:])
            nc.sync.dma_start(bk2_sb[:], b_k2[:])
            nc.sync.dma_start(bk1_sb[:], b_k1[:])
            nc.sync.dma_start(bq2_sb[:], b_q2[:])
            nc.sync.dma_start(bv1_sb[:], b_v1[:])
            nc.sync.dma_start(bv2_sb[:], b_v2[:])
            nc.sync.dma_start(idn_sb[:], idn[:])

            def load_w(wp, name):
                w_sb = p_w.tile([P, DT, D], BF16, tag="w", name=name)
                nc.sync.dma_start(w_sb[:], wp[:])
                return w_sb

            def proj_T(w_sb, src_sb, bias_sb, name, odt):
                """out[e, n] = sum_d W[e,d] src[n,d] + b[e], e-major."""
                o_sb = p_qk.tile([P, DT, N], odt, tag="qk", name=name)
                for eo in range(DT):
                    ps = p_ps.tile([P, 4, CH], F32, tag="ps")
                    for do in range(DT):
                        for ch in range(NCH):
                            nc.tensor.matmul(
                                ps[:, ch, :],
                                w_sb[:, do, eo * P : (eo + 1) * P],
                                src_sb[:, do, ch * CH : (ch + 1) * CH],
                                start=(do == 0),
                                stop=(do == DT - 1),
                            )
                    nc.scalar.activation(
                        o_sb[:, eo, :].rearrange("p (m c) -> p m c", m=4),
                        ps[:],
                        mybir.ActivationFunctionType.Identity,
                        bias=bias_sb[:, eo : eo + 1],
                    )
                return o_sb

            def proj_V(w_sb, src_sb, bias_sb, name):
                """out[m, e] = sum_d src[m,d] W[e,d] + b[e], m-major bf16."""
                o_sb = p_v.tile([P, NT, D], BF16, tag="v", name=name)
                for mq in range(NT // 4):
                    ps = p_ps.tile([P, 4, CH], F32, tag="ps")
                    for mi in range(4):
                        mt = 4 * mq + mi
                        for do in range(DT):
                            nc.tensor.matmul(
                                ps[:, mi, :],
                                src_sb[:, do, mt * P : (mt + 1) * P],
                                w_sb[:, do, :],
                                start=(do == 0),
                                stop=(do == DT - 1),
                            )
                    nc.vector.tensor_add(
                        out=o_sb[:, 4 * mq : 4 * mq + 4, :],
                        in0=ps[:],
                        in1=bias_sb[:],
                    )
                return o_sb

            DR = mybir.MatmulPerfMode.DoubleRow

            def scores_exp(kt_sb, qt_sb, es_dram, chs):
                """Es[ch, mt, p, c] = exp(sum_e K[m,e] Q[n,e]) - c -> fp8 DRAM."""
                for ch in chs:
                    csl = slice(ch * CH, (ch + 1) * CH)
                    for mq in range(NT // 4):
                        ps = p_ps.tile([P, 4, CH], F32, tag="ps")
                        for mi in range(4):
                            mt = 4 * mq + mi
                            for eo in range(DT):
                                nc.tensor.matmul(
                                    ps[:, mi, :],
                                    kt_sb[:, eo, mt * P : (mt + 1) * P],
                                    qt_sb[:, eo, csl],
                                    start=(eo == 0),
                                    stop=(eo == DT - 1),
                                )
                        e_sb = p_esb.tile([P, 4, CH], BF16, tag="esb")
                        nc.scalar.activation(
                            e_sb[:], ps[:], mybir.ActivationFunctionType.Exp
                        )
                        es_sb = p_es8.tile([P, 4, CH], F8, tag="es8")
                        nc.vector.tensor_scalar_sub(
                            out=es_sb[:], in0=e_sb[:], scalar1=C_SHIFT
                        )
                        nc.sync.dma_start(
                            es_dram[ch, 4 * mq : 4 * mq + 4].rearrange(
                                "m p c -> p m c"
                            ),
                            es_sb[:],
                        )

            def ar_half(es_d, z_halves, h):
                sl = slice(2 * h, 2 * h + 2)
                nc.gpsimd.collective_compute(
                    "AllReduce",
                    mybir.AluOpType.add,
                    replica_groups=[list(range(NCORES))],
                    ins=[es_d[sl].opt()],
                    outs=[z_halves[h][:].opt()],
                )

            # ======== branch 1: Q1/K2 projections, E1 scores, ARs ========
            q1t = proj_T(w1_sb, xt_sb, bq1_sb, "q1t", BF16)
            k2t = proj_T(w2_sb, yt_sb, bk2_sb, "k2t", BF16)
            scores_exp(k2t, q1t, e1s_d, (0, 1))
            ar_half(e1s_d, z1_h, 0)
            scores_exp(k2t, q1t, e1s_d, (2, 3))
            ar_half(e1s_d, z1_h, 1)

            # ======== branch 2: K1/Q2 projections, E2 scores, ARs ========
            w_sb = load_w(w_k1, "wk1")
            k1t = proj_T(w_sb, xt_sb, bk1_sb, "k1t", BF16)
            w_sb = load_w(w_q2, "wq2")
            q2t = proj_T(w_sb, yt_sb, bq2_sb, "q2t", BF16)
            scores_exp(k1t, q2t, e2s_d, (0, 1))
            ar_half(e2s_d, z2_h, 0)
            scores_exp(k1t, q2t, e2s_d, (2, 3))
            ar_half(e2s_d, z2_h, 1)

            # ======== V projections ========
            w_sb = load_w(w_v2, "wv2")
            v2 = proj_V(w_sb, yt_sb, bv2_sb, "v2")
            w_sb = load_w(w_v1, "wv1")
            v1 = proj_V(w_sb, xt_sb, bv1_sb, "v1")

            # ======== phase C ========
            def make_a_half(es_d, z_src, ch, h, name):
                """A[:, 8 mt, c] = (Es + c)/Z for chunk ch, half h; bf16."""
                msl = slice(h * HT, (h + 1) * HT)
                z_b = p_zb.tile([P, HT, CH], F8, tag="zb", name=f"zb{name}")
                nc.sync.dma_start(z_b[:], z_src(ch, msl))
                z_f = p_zf.tile([P, HT, CH], F32, tag="zf", name=f"zf{name}")
                nc.vector.tensor_scalar_add(out=z_f[:], in0=z_b[:], scalar1=Z_BIAS)
                rz = p_rz.tile([P, HT, CH], F32, tag="rz", name=f"rz{name}")
                nc.vector.reciprocal_approx_fast(out=rz[:], in_=z_f[:])
                e_b = p_ere.tile([P, HT, CH], F8, tag="ere", name=f"eb{name}")
                nc.sync.dma_start(e_b[:], es_d[ch, msl].rearrange("m p c -> p m c"))
                a_sb = p_a.tile([P, HT, CH], BF16, tag="a", name=f"a{name}")
                nc.vector.scalar_tensor_tensor(
                    out=a_sb[:],
                    in0=e_b[:],
                    scalar=C_SHIFT,
                    in1=rz[:],
                    op0=mybir.AluOpType.add,
                    op1=mybir.AluOpType.mult,
                )
                return a_sb

            def z1_src(ch, msl):
                return z1_h[ch // 2][ch % 2, msl].rearrange("m p c -> p m c")

            def z2_src(ch, msl):
                return z2_h[ch // 2][ch % 2, msl].rearrange("m p c -> p m c")

            ps_held = {}

            def u1_pass(ch):
                a1 = [make_a_half(e1s_d, z1_src, ch, h, f"1{h}") for h in range(2)]
                ps = p_ps.tile([P, 4, CH], F32, tag="ps")
                for dt in range(DT):
                    dsl = slice(dt * P, (dt + 1) * P)
                    for mt in range(NT):
                        nc.tensor.matmul(
                            ps[:, dt, :],
                            v2[:, mt, dsl],
                            a1[mt // HT][:, mt % HT, :],
                            start=(mt == 0),
                            stop=False,
                        )
                ps_held[ch] = ps

            def u2_pass(ch):
                a2 = [make_a_half(e2s_d, z2_src, ch, h, f"2{h}") for h in range(2)]
                csl = slice(ch * CH, (ch + 1) * CH)
                ps = ps_held[ch]
                for dt in range(DT):
                    dsl = slice(dt * P, (dt + 1) * P)
                    psd = ps[:, dt, :]
                    for mt in range(NT):
                        nc.tensor.matmul(
                            psd,
                            v1[:, mt, dsl],
                            a2[mt // HT][:, mt % HT, :],
                            start=False,
                            stop=False,
                        )
                    # residual: out += I.T @ (X^T + Y^T) chunk
                    nc.tensor.matmul(psd, idn_sb[:], xt_sb[:, dt, csl],
                                     start=False, stop=False)
                    nc.tensor.matmul(psd, idn_sb[:], yt_sb[:, dt, csl],
                                     start=False, stop=True)
                ot = p_ot.tile([P, 4, CH], BF16, tag="ot")
                nc.scalar.activation(
                    ot[:], ps[:], mybir.ActivationFunctionType.Copy
                )
                nc.sync.dma_start(
                    out[:, ch].rearrange("d p c -> p d c"), ot[:]
                )

            u1_pass(0)
            u1_pass(1)
            u2_pass(0)
            u1_pass(2)
            u2_pass(1)
            u1_pass(3)
            u2_pass(2)
            u2_pass(3)

    nc.compile()
    return nc


def _pmajor(a, inner):
    """[O*P, F] -> [P, O, F] partition-major."""
    o = a.shape[0] // inner
    return np.ascontiguousarray(a.reshape(o, inner, a.shape[1]).transpose(1, 0, 2))


def _prep_inputs(inputs):
    import ml_dtypes

    X = np.asarray(inputs["X"], dtype=np.float32)
    Y = np.asarray(inputs["Y"], dtype=np.float32)
    shalf = np.float32(1.0 / np.sqrt(np.sqrt(D)))  # D^-1/4 into both Q and K

    def wT(name, s=np.float32(1.0)):
        w = np.asarray(inputs[f"W_{name}"], dtype=np.float32)
        return _pmajor((w.T * s).astype(ml_dtypes.bfloat16), P)

    def bstripe(name, s=np.float32(1.0)):
        b = np.asarray(inputs[f"b_{name}"], dtype=np.float32) * s
        return np.ascontiguousarray(b.reshape(DT, P).T)

    def bbcast4(name):
        b = np.asarray(inputs[f"b_{name}"], dtype=np.float32).astype(ml_dtypes.bfloat16)
        return np.ascontiguousarray(np.broadcast_to(b, (P, 4, D)))

    shared = {
        "WQ1T": wT("xq", shalf),
        "WK1T": wT("xk", shalf),
        "WV1T": wT("xv"),
        "WQ2T": wT("yq", shalf),
        "WK2T": wT("yk", shalf),
        "WV2T": wT("yv"),
        "BQ1": bstripe("xq", shalf),
        "BK1": bstripe("xk", shalf),
        "BQ2": bstripe("yq", shalf),
        "BK2": bstripe("yk", shalf),
        "BV1": bbcast4("xv"),
        "BV2": bbcast4("yv"),
        "IDN": np.eye(P, dtype=ml_dtypes.bfloat16),
    }
    in_maps = []
    for c in range(NCORES):
        xt = np.ascontiguousarray(X[c].T)
        yt = np.ascontiguousarray(Y[c].T)
        m = dict(shared)
        m["XTB"] = _pmajor(xt.astype(ml_dtypes.bfloat16), P)
        m["YTB"] = _pmajor(yt.astype(ml_dtypes.bfloat16), P)
        in_maps.append(m)
    return in_maps


def _unblock(ot):
    """[DT, NCH, P, CH] -> [N, D] (transposed back)."""
    return ot.transpose(0, 2, 1, 3).reshape(D, N).T


def kernel(**inputs):
    if "nc" not in _CACHE:
        _CACHE["nc"] = build()
    nc = _CACHE["nc"]
    in_maps = _prep_inputs(inputs)
    res = run_bass_kernel_spmd(
        nc, in_maps, core_ids=list(range(NCORES)), **_CACHE.get("run_kwargs", {})
    )
    _CACHE["last_result"] = res
    out = np.stack(
        [np.ascontiguousarray(_unblock(res.results[c]["OT"])) for c in range(NCORES)]
    )
    return out.astype(np.float32)


# revision 8
# speedup vs baseline: 2.6811x; 1.0128x over previous
"""Trainium2 Bass kernel for nn_CA_82300163326040.

Cross-attention between X and Y with softmax over the BATCH axis (torch
legacy dim=0). B=8, N=2048, D=512, f32.

Sharding: data-parallel over batch, one batch per NeuronCore (8 cores).
The batch-axis softmax couples cores: each core computes its local
shifted exp-scores Es = exp(Q.K^T*s) - c ([2048,2048], fp8e4m3) and the
denominators Z = sum_b E are obtained with 8-core fp8 AllReduces of Es
(Z = AR(Es) + 8c reconstructed on the fly). The shift c ~ E[exp(S)]
keeps |Es| ~3x smaller than E so fp8 quantization + the mesh CCE's
fp8 partial-sum roundings stay small. No max-subtraction is needed:
S ~ N(0, 1/9) so exp() cannot overflow.

v3 (vs 579us baseline / 547us v2a):
  - Q/K projections emit fp8e4m3 (D^-1/4 folded into both Q and K
    weights so values sit in fp8's normal range); score matmuls run
    fp8 DoubleRow (2 k-subtiles per instruction, ~1.7x).
  - Single fp8 score tensor: AR input IS the stored numerator; phase C
    reconstructs A = (Es + c) * (1/Z) in one fused DVE
    scalar_tensor_tensor. No bf16 E round-trip (saves 32MB DMA).
  - 4-bank-wide PSUM tiles ([128,2048]) everywhere: 4x fewer ACT
    evictions/DMAs, each amortizing the 352-cycle ACT ramp.
  - X+Y residual accumulated in PSUM via identity matmuls; bf16 output.
  - Z fp8->f32 (+8c) conversion on the otherwise-idle GpSimd engine.
  - E1 scores + their ARs issue right after the Q1/K2 projections, so
    all four AllReduces hide under compute.
"""

import numpy as np

import concourse.bass as bass
import concourse.mybir as mybir
import concourse.tile as tile
from concourse import bacc
from concourse.bass_utils import run_bass_kernel_spmd

P = 128
N = 2048  # sequence length
D = 512  # model dim
NCORES = 8
DT = D // P  # 4 feature tiles
NT = N // P  # 16 sequence tiles
CH = 512  # n-chunk (free dim of all matmuls)
NCH = N // CH  # 4 chunks
HT = NT // 2  # half of the m-tiles (phase-C staging granularity)
WF = 4 * CH  # wide psum tile free size (4 banks)

F32 = mybir.dt.float32
BF16 = mybir.dt.bfloat16
F8 = mybir.dt.float8e4

C_SHIFT = 1.0546875  # ~ E[exp(S)]; Es = E - c
Z_BIAS = 8 * C_SHIFT  # Z = AR(Es) + 8c

_CACHE = {}


def build():
    nc = bacc.Bacc("TRN2", target_bir_lowering=False, debug=False, num_devices=NCORES)

    # ---- parameters (per core), all pre-arranged on host ----
    xtb = nc.declare_dram_parameter("XTB", [P, DT, N], BF16, isOutput=False)
    ytb = nc.declare_dram_parameter("YTB", [P, DT, N], BF16, isOutput=False)
    # weights (transposed, partition-major): w[p, o, e] = W^T[o*128+p, e]
    w_q1 = nc.declare_dram_parameter("WQ1T", [P, DT, D], BF16, isOutput=False)
    w_k1 = nc.declare_dram_parameter("WK1T", [P, DT, D], BF16, isOutput=False)
    w_v1 = nc.declare_dram_parameter("WV1T", [P, DT, D], BF16, isOutput=False)
    w_q2 = nc.declare_dram_parameter("WQ2T", [P, DT, D], BF16, isOutput=False)
    w_k2 = nc.declare_dram_parameter("WK2T", [P, DT, D], BF16, isOutput=False)
    w_v2 = nc.declare_dram_parameter("WV2T", [P, DT, D], BF16, isOutput=False)
    b_q1 = nc.declare_dram_parameter("BQ1", [P, DT], F32, isOutput=False)
    b_k1 = nc.declare_dram_parameter("BK1", [P, DT], F32, isOutput=False)
    b_q2 = nc.declare_dram_parameter("BQ2", [P, DT], F32, isOutput=False)
    b_k2 = nc.declare_dram_parameter("BK2", [P, DT], F32, isOutput=False)
    b_v1 = nc.declare_dram_parameter("BV1", [P, 4, D], BF16, isOutput=False)  # x4 tiled
    b_v2 = nc.declare_dram_parameter("BV2", [P, 4, D], BF16, isOutput=False)
    idn = nc.declare_dram_parameter("IDN", [P, P], BF16, isOutput=False)

    out = nc.declare_dram_parameter("OT", [DT, NCH, P, CH], BF16, isOutput=True)

    with tile.TileContext(nc) as tc:
        with (
            tc.tile_pool(name="w", bufs=2) as p_w,
            tc.tile_pool(name="bias", bufs=1) as p_bias,
            tc.tile_pool(name="inp", bufs=2) as p_in,
            tc.tile_pool(name="qk", bufs=2) as p_qk,
            tc.tile_pool(name="v", bufs=2) as p_v,
            tc.tile_pool(name="zf", bufs=1) as p_zf,
            tc.tile_pool(name="rz", bufs=1) as p_rz,
            tc.tile_pool(name="zb", bufs=2) as p_zb,
            tc.tile_pool(name="ere", bufs=2) as p_ere,
            tc.tile_pool(name="a", bufs=3) as p_a,
            tc.tile_pool(name="esb", bufs=2) as p_esb,
            tc.tile_pool(name="es8", bufs=2) as p_es8,
            tc.tile_pool(name="ot", bufs=2) as p_ot,
            tc.tile_pool(name="ps", bufs=2, space="PSUM") as p_ps,
            tc.tile_pool(name="dram", bufs=1, space="DRAM") as p_dram,
        ):
            # ---- DRAM intermediates: shifted fp8 scores + AR outputs ----
            e1s_d = p_dram.tile([NCH, NT, P, CH], F8, tag="e1s")
            e2s_d = p_dram.tile([NCH, NT, P, CH], F8, tag="e2s")
            z1_h = [
                p_dram.tile([2, NT, P, CH], F8, tag=f"z1{h}",
                            addr_space="Shared", name=f"z1{h}")
                for h in range(2)
            ]
            z2_h = [
                p_dram.tile([2, NT, P, CH], F8, tag=f"z2{h}",
                            addr_space="Shared", name=f"z2{h}")
                for h in range(2)
            ]

            # ---- resident loads; first-proj operands issued first ----
            w1_sb = p_w.tile([P, DT, D], BF16, tag="w", name="wq1")
            nc.sync.dma_start(w1_sb[:], w_q1[:])
            xt_sb = p_in.tile([P, DT, N], BF16, tag="in", name="xt")
            nc.sync.dma_start(xt_sb[:, :, 0:1024], xtb[:, :, 0:1024])
            nc.sync.dma_start(xt_sb[:, :, 1024:2048], xtb[:, :, 1024:2048])
            w2_sb = p_w.tile([P, DT, D], BF16, tag="w", name="wk2")
            nc.sync.dma_start(w2_sb[:], w_k2[:])
            yt_sb = p_in.tile([P, DT, N], BF16, tag="in", name="yt")
            nc.sync.dma_start(yt_sb[:, :, 0:1024], ytb[:, :, 0:1024])
            nc.sync.dma_start(yt_sb[:, :, 1024:2048], ytb[:, :, 1024:2048])

            bq1_sb = p_bias.tile([P, DT], F32, tag="bq1")
            bk1_sb = p_bias.tile([P, DT], F32, tag="bk1")
            bq2_sb = p_bias.tile([P, DT], F32, tag="bq2")
            bk2_sb = p_bias.tile([P, DT], F32, tag="bk2")
            bv1_sb = p_bias.tile([P, 4, D], BF16, tag="bv1")
            bv2_sb = p_bias.tile([P, 4, D], BF16, tag="bv2")
            idn_sb = p_bias.tile([P, P], BF16, tag="idn")
            nc.sync.dma_start(bq1_sb[:], b_q1[:])
            nc.sync.dma_start(bk2_sb[:], b_k2[:])
            nc.sync.dma_start(bk1_sb[:], b_k1[:])
            nc.sync.dma_start(bq2_sb[:], b_q2[:])
            nc.sync.dma_start(bv1_sb[:], b_v1[:])
            nc.sync.dma_start(bv2_sb[:], b_v2[:])
            nc.sync.dma_start(idn_sb[:], idn[:])

            def load_w(wp, name):
                w_sb = p_w.tile([P, DT, D], BF16, tag="w", name=name)
                nc.scalar.dma_start(w_sb[:], wp[:])
                return w_sb

            def proj_T(w_sb, src_sb, bias_sb, name, odt):
                """out[e, n] = sum_d W[e,d] src[n,d] + b[e], e-major."""
                o_sb = p_qk.tile([P, DT, N], odt, tag="qk", name=name)
                for eo in range(DT):
                    ps = p_ps.tile([P, 4, CH], F32, tag="ps")
                    for do in range(DT):
                        for ch in range(NCH):
                            nc.tensor.matmul(
                                ps[:, ch, :],
                                w_sb[:, do, eo * P : (eo + 1) * P],
                                src_sb[:, do, ch * CH : (ch + 1) * CH],
                                start=(do == 0),
                                stop=(do == DT - 1),
                            )
                    nc.scalar.activation(
                        o_sb[:, eo, :].rearrange("p (m c) -> p m c", m=4),
                        ps[:],
                        mybir.ActivationFunctionType.Identity,
                        bias=bias_sb[:, eo : eo + 1],
                    )
                return o_sb

            def proj_V(w_sb, src_sb, bias_sb, name):
                """out[m, e] = sum_d src[m,d] W[e,d] + b[e], m-major bf16."""
                o_sb = p_v.tile([P, NT, D], BF16, tag="v", name=name)
                for mq in range(NT // 4):
                    ps = p_ps.tile([P, 4, CH], F32, tag="ps")
                    for mi in range(4):
                        mt = 4 * mq + mi
                        for do in range(DT):
                            nc.tensor.matmul(
                                ps[:, mi, :],
                                src_sb[:, do, mt * P : (mt + 1) * P],
                                w_sb[:, do, :],
                                start=(do == 0),
                                stop=(do == DT - 1),
                            )
                    nc.vector.tensor_add(
                        out=o_sb[:, 4 * mq : 4 * mq + 4, :],
                        in0=ps[:],
                        in1=bias_sb[:],
                    )
                return o_sb

            DR = mybir.MatmulPerfMode.DoubleRow

            def scores_exp(kt_sb, qt_sb, es_dram, chs):
                """Es[ch, mt, p, c] = exp(sum_e K[m,e] Q[n,e]) - c -> fp8 DRAM."""
                for ch in chs:
                    csl = slice(ch * CH, (ch + 1) * CH)
                    for mq in range(NT // 4):
                        ps = p_ps.tile([P, 4, CH], F32, tag="ps")
                        for mi in range(4):
                            mt = 4 * mq + mi
                            for eo in range(DT):
                                nc.tensor.matmul(
                                    ps[:, mi, :],
                                    kt_sb[:, eo, mt * P : (mt + 1) * P],
                                    qt_sb[:, eo, csl],
                                    start=(eo == 0),
                                    stop=(eo == DT - 1),
                                )
                        e_sb = p_esb.tile([P, 4, CH], BF16, tag="esb")
                        nc.scalar.activation(
                            e_sb[:], ps[:], mybir.ActivationFunctionType.Exp
                        )
                        es_sb = p_es8.tile([P, 4, CH], F8, tag="es8")
                        nc.vector.tensor_scalar_sub(
                            out=es_sb[:], in0=e_sb[:], scalar1=C_SHIFT
                        )
                        nc.sync.dma_start(
                            es_dram[ch, 4 * mq : 4 * mq + 4].rearrange(
                                "m p c -> p m c"
                            ),
                            es_sb[:],
                        )

            def ar_half(es_d, z_halves, h):
                sl = slice(2 * h, 2 * h + 2)
                nc.gpsimd.collective_compute(
                    "AllReduce",
                    mybir.AluOpType.add,
                    replica_groups=[list(range(NCORES))],
                    ins=[es_d[sl].opt()],
                    outs=[z_halves[h][:].opt()],
                )

            # ======== branch 1: Q1/K2 projections, E1 scores, ARs ========
            q1t = proj_T(w1_sb, xt_sb, bq1_sb, "q1t", BF16)
            k2t = proj_T(w2_sb, yt_sb, bk2_sb, "k2t", BF16)
            scores_exp(k2t, q1t, e1s_d, (0, 1))
            wk1_sb = load_w(w_k1, "wk1")
            ar_half(e1s_d, z1_h, 0)
            scores_exp(k2t, q1t, e1s_d, (2, 3))
            wq2_sb = load_w(w_q2, "wq2")
            ar_half(e1s_d, z1_h, 1)

            # ======== branch 2: K1/Q2 projections, E2 scores, ARs ========
            k1t = proj_T(wk1_sb, xt_sb, bk1_sb, "k1t", BF16)
            q2t = proj_T(wq2_sb, yt_sb, bq2_sb, "q2t", BF16)
            scores_exp(k1t, q2t, e2s_d, (0, 1))
            wv2_sb = load_w(w_v2, "wv2")
            ar_half(e2s_d, z2_h, 0)
            scores_exp(k1t, q2t, e2s_d, (2, 3))
            wv1_sb = load_w(w_v1, "wv1")
            ar_half(e2s_d, z2_h, 1)

            # ======== V projections ========
            v2 = proj_V(wv2_sb, yt_sb, bv2_sb, "v2")
            v1 = proj_V(wv1_sb, xt_sb, bv1_sb, "v1")

            # ======== phase C ========
            def make_a_half(es_d, z_src, ch, h, name):
                """A[:, 8 mt, c] = (Es + c)/Z for chunk ch, half h; bf16."""
                msl = slice(h * HT, (h + 1) * HT)
                z_b = p_zb.tile([P, HT, CH], F8, tag="zb", name=f"zb{name}")
                nc.sync.dma_start(z_b[:], z_src(ch, msl))
                z_f = p_zf.tile([P, HT, CH], F32, tag="zf", name=f"zf{name}")
                nc.scalar.activation(
                    z_f[:], z_b[:], mybir.ActivationFunctionType.Copy, bias=Z_BIAS
                )
                rz = p_rz.tile([P, HT, CH], F32, tag="rz", name=f"rz{name}")
                nc.vector.reciprocal_approx_fast(out=rz[:], in_=z_f[:])
                e_b = p_ere.tile([P, HT, CH], F8, tag="ere", name=f"eb{name}")
                nc.sync.dma_start(e_b[:], es_d[ch, msl].rearrange("m p c -> p m c"))
                a_sb = p_a.tile([P, HT, CH], BF16, tag="a", name=f"a{name}")
                nc.vector.scalar_tensor_tensor(
                    out=a_sb[:],
                    in0=e_b[:],
                    scalar=C_SHIFT,
                    in1=rz[:],
                    op0=mybir.AluOpType.add,
                    op1=mybir.AluOpType.mult,
                )
                return a_sb

            def z1_src(ch, msl):
                return z1_h[ch // 2][ch % 2, msl].rearrange("m p c -> p m c")

            def z2_src(ch, msl):
                return z2_h[ch // 2][ch % 2, msl].rearrange("m p c -> p m c")

            ps_held = {}

            def u1_pass(ch):
                a1 = [make_a_half(e1s_d, z1_src, ch, h, f"1{h}") for h in range(2)]
                ps = p_ps.tile([P, 4, CH], F32, tag="ps")
                for dt in range(DT):
                    dsl = slice(dt * P, (dt + 1) * P)
                    for mt in range(NT):
                        nc.tensor.matmul(
                            ps[:, dt, :],
                            v2[:, mt, dsl],
                            a1[mt // HT][:, mt % HT, :],
                            start=(mt == 0),
                            stop=False,
                        )
                ps_held[ch] = ps

            def u2_pass(ch):
                a2 = [make_a_half(e2s_d, z2_src, ch, h, f"2{h}") for h in range(2)]
                csl = slice(ch * CH, (ch + 1) * CH)
                ps = ps_held[ch]
                for dt in range(DT):
                    dsl = slice(dt * P, (dt + 1) * P)
                    psd = ps[:, dt, :]
                    for mt in range(NT):
                        nc.tensor.matmul(
                            psd,
                            v1[:, mt, dsl],
                            a2[mt // HT][:, mt % HT, :],
                            start=False,
                            stop=False,
                        )
                    # residual: out += I.T @ (X^T + Y^T) chunk
                    nc.tensor.matmul(psd, idn_sb[:], xt_sb[:, dt, csl],
                                     start=False, stop=False)
                    nc.tensor.matmul(psd, idn_sb[:], yt_sb[:, dt, csl],
                                     start=False, stop=True)
                    ot = p_ot.tile([P, CH], BF16, tag="ot")
                    nc.scalar.activation(
                        ot[:], psd, mybir.ActivationFunctionType.Copy
                    )
                    nc.sync.dma_start(out[dt, ch], ot[:])

            u1_pass(0)
            u1_pass(1)
            u2_pass(0)
            u1_pass(2)
            u2_pass(1)
            u1_pass(3)
            u2_pass(2)
            u2_pass(3)

    nc.compile()
    return nc


def _pmajor(a, inner):
    """[O*P, F] -> [P, O, F] partition-major."""
    o = a.shape[0] // inner
    return np.ascontiguousarray(a.reshape(o, inner, a.shape[1]).transpose(1, 0, 2))


def _prep_inputs(inputs):
    import ml_dtypes

    X = np.asarray(inputs["X"], dtype=np.float32)
    Y = np.asarray(inputs["Y"], dtype=np.float32)
    shalf = np.float32(1.0 / np.sqrt(np.sqrt(D)))  # D^-1/4 into both Q and K

    def wT(name, s=np.float32(1.0)):
        w = np.asarray(inputs[f"W_{name}"], dtype=np.float32)
        return _pmajor((w.T * s).astype(ml_dtypes.bfloat16), P)

    def bstripe(name, s=np.float32(1.0)):
        b = np.asarray(inputs[f"b_{name}"], dtype=np.float32) * s
        return np.ascontiguousarray(b.reshape(DT, P).T)

    def bbcast4(name):
        b = np.asarray(inputs[f"b_{name}"], dtype=np.float32).astype(ml_dtypes.bfloat16)
        return np.ascontiguousarray(np.broadcast_to(b, (P, 4, D)))

    shared = {
        "WQ1T": wT("xq", shalf),
        "WK1T": wT("xk", shalf),
        "WV1T": wT("xv"),
        "WQ2T": wT("yq", shalf),
        "WK2T": wT("yk", shalf),
        "WV2T": wT("yv"),
        "BQ1": bstripe("xq", shalf),
        "BK1": bstripe("xk", shalf),
        "BQ2": bstripe("yq", shalf),
        "BK2": bstripe("yk", shalf),
        "BV1": bbcast4("xv"),
        "BV2": bbcast4("yv"),
        "IDN": np.eye(P, dtype=ml_dtypes.bfloat16),
    }
    in_maps = []
    for c in range(NCORES):
        xt = np.ascontiguousarray(X[c].T)
        yt = np.ascontiguousarray(Y[c].T)
        m = dict(shared)
        m["XTB"] = _pmajor(xt.astype(ml_dtypes.bfloat16), P)
        m["YTB"] = _pmajor(yt.astype(ml_dtypes.bfloat16), P)
        in_maps.append(m)
    return in_maps


def _unblock(ot):
    """[DT, NCH, P, CH] -> [N, D] (transposed back)."""
    return ot.transpose(0, 2, 1, 3).reshape(D, N).T


def kernel(**inputs):
    if "nc" not in _CACHE:
        _CACHE["nc"] = build()
    nc = _CACHE["nc"]
    in_maps = _prep_inputs(inputs)
    res = run_bass_kernel_spmd(
        nc, in_maps, core_ids=list(range(NCORES)), **_CACHE.get("run_kwargs", {})
    )
    _CACHE["last_result"] = res
    out = np.stack(
        [np.ascontiguousarray(_unblock(res.results[c]["OT"])) for c in range(NCORES)]
    )
    return out.astype(np.float32)
